# revision 19
# baseline (speedup 1.0000x reference)
"""Bass/Trainium2 kernel for a 2-layer bidirectional QRNN (fo-pooling).

Reference computation (per layer, per direction):
    ZFO = X @ W + b            # [S, B, 3H]
    Z, F, O = split(ZFO); Z = tanh(Z); F = sigmoid(F); O = sigmoid(O)
    c_t = F_t * c_{t-1} + (1 - F_t) * Z_t        (bw direction: reversed time)
    Y_dir = O * C
    Y = concat(Y_fw, Y_bw)     # [S, B, 2H]
Two stacked layers; output is [B, S, 2H].

Sharding: data-parallel over batch. B=16 rows -> 2 rows per NeuronCore x 8.
Each core runs both layers for its 2 rows; no collectives.

v2 design (635us -> target ~PE roofline):
- All matmul operands are fp16 (1 cyc/row on the PE, same as fp32r, but
  2-byte LDWEIGHTS and no compute-producer restriction: weights and X are
  host-pre-cast and DMA'd directly; the fp32r staging/cast machinery and the
  192 DVE input casts of v1 are gone). Whole-pipeline fp16 rel-err vs the
  fp32 reference is ~1.4e-3 (CPU-simulated exactly).
- X ([BC,D,S] fp16, 4MB/core) and the layer-0 output y1 ([BC,2H,S] fp16,
  8MB/core) are SBUF-resident; y1 never round-trips DRAM. Total HBM traffic
  drops ~103MB -> ~21MB/core, and DMA issues from 348 -> ~80.
- Gate combine fused: one DVE scalar_tensor_tensor computes g' = (f-1)*z and
  the DVE scan runs state = f*state - g' = f*state + (1-f)*z, so the
  elementwise work per [128,512] tile is 3 scalar activations + 2 DVE ops +
  1 gpsimd mul -- all hidden under the 12 (layer0) / 24 (layer1) matmuls.
- Pass order l0-fw, l0-bw, l1-bw, l1-fw: l1-bw consumes y1 s-tiles in the
  order l0-bw produces them (descending), and l1-fw's first s-tile needs
  l0-bw's last -- so the PE queue never waits at a pass boundary.
"""

import numpy as np

import concourse.bacc as bacc
import concourse.mybir as mybir
from concourse import bass_utils
from concourse.tile import TileContext

# problem dims (hardcoded per spec)
B, S, D, H = 16, 2048, 512, 512
N_CORES = 8
BC = B // N_CORES  # batch rows per core
P = 128            # SBUF partitions
ST = 512           # s-tile (max moving free dim)
NS = S // ST       # 4 s-tiles
HC = H // P        # 4 hidden chunks
K0 = D // P        # 4 contraction chunks, layer 0
K1 = 2 * H // P    # 8 contraction chunks, layer 1

F32 = mybir.dt.float32
F16 = mybir.dt.float16
ACT = mybir.ActivationFunctionType
ALU = mybir.AluOpType


def build_nc():
    """Build the SPMD Bass program (same program on every core)."""
    nc = bacc.Bacc("TRN2", target_bir_lowering=False)

    xt = nc.dram_tensor("xt", [BC, K0, P, S], F16, kind="ExternalInput")
    w0f = nc.dram_tensor("w0f", [K0, P, 3 * H], F16, kind="ExternalInput")
    w0b = nc.dram_tensor("w0b", [K0, P, 3 * H], F16, kind="ExternalInput")
    w1f = nc.dram_tensor("w1f", [K1, P, 3 * H], F16, kind="ExternalInput")
    w1b = nc.dram_tensor("w1b", [K1, P, 3 * H], F16, kind="ExternalInput")
    # biases host-prearranged to [P, 12] so the load is a plain 2D DMA (the
    # "(j p) -> p j" gather was 4-byte-element descriptors, ~11.5us!)
    b0f = nc.dram_tensor("b0f", [P, 3 * HC], F32, kind="ExternalInput")
    b0b = nc.dram_tensor("b0b", [P, 3 * HC], F32, kind="ExternalInput")
    b1f = nc.dram_tensor("b1f", [P, 3 * HC], F32, kind="ExternalInput")
    b1b = nc.dram_tensor("b1b", [P, 3 * HC], F32, kind="ExternalInput")
    # out rows indexed (dir*HC + hh)*P + p -> feature dir*H + hh*P + p
    out_t = nc.dram_tensor("out_t", [BC, 2 * HC, P, S], F16, kind="ExternalOutput")

    with TileContext(nc) as tc:
        with (
            tc.tile_pool(name="res", bufs=1) as rpool,
            tc.tile_pool(name="scr", bufs=3) as spool,
            tc.tile_pool(name="carry", bufs=1) as cpool,
            tc.tile_pool(name="ps", bufs=1, space="PSUM") as ppool,
        ):
            # ---- resident loads: X rides the sync HWDGE queue, weights +
            # biases the scalar HWDGE queue, so the two DMA streams transfer
            # in parallel and the first matmul is gated by max(X[b0,s-half0],
            # w0f) instead of their sum. Per-k chunks (contiguous 2D
            # patterns, cheap ~0.6us issues); X additionally split into
            # s-halves so iteration 0 only waits for the first half. The
            # scalar queue's issue burst (28 x 0.6us) finishes before the
            # first activation needs it. ----
            def wload(wd, kc, name, eng):
                wt = rpool.tile([P, kc, 3 * H], F16, name=name)
                for k in range(kc):
                    eng.dma_start(wt[:, k, :], wd[k, :, :])
                return wt

            btiles = {n: rpool.tile([P, 3 * HC], F32, name=n)
                      for n in ("bt0f", "bt0b", "bt1f", "bt1b")}

            def bload(bd, name, eng):
                bt = btiles[name]
                eng.dma_start(bt[:], bd[:, :])
                return bt

            xres = [[rpool.tile([P, S], F16, name=f"x{b}k{k}") for k in range(K0)]
                    for b in range(BC)]

            def xload(b, half):
                sl = slice(0, S // 2) if half == 0 else slice(S // 2, S)
                for k in range(K0):
                    nc.sync.dma_start(xres[b][k][:, sl], xt[b, k, :, sl])

            # The first matmul iteration is gated on X[b0, s-half0] + all of
            # w0f: balance those ~2.5MB across BOTH HWDGE queues so they
            # transfer in parallel (~4us), then keep the scalar queue clear
            # for activations. Everything else queues on sync behind X --
            # transferred long before use (w0b by ~35us vs first use ~140us),
            # and out-writes only start at layer 1 (~300us).
            wt0f = rpool.tile([P, K0, 3 * H], F16, name="wt0f")
            nc.scalar.dma_start(wt0f[:, 0, :], w0f[0, :, :])
            nc.scalar.dma_start(wt0f[:, 1, :], w0f[1, :, :])
            bt0f = bload(b0f, "bt0f", nc.scalar)
            # warmup activations: force BOTH act-table loads (tanh+sigmoid,
            # 1.3us each) onto the scalar queue now, during the X-transfer
            # window -- otherwise the sigmoid table loads lazily between the
            # first real tanh and sigmoid. Outputs land in bt0b, which the
            # (later-issued) b0b DMA fully overwrites.
            nc.scalar.activation(btiles["bt0b"][:, 0:1], bt0f[:, 0:1], ACT.Tanh)
            nc.scalar.activation(btiles["bt0b"][:, 1:2], bt0f[:, 0:1], ACT.Sigmoid)
            xload(0, 0)
            nc.sync.dma_start(wt0f[:, 2, :], w0f[2, :, :])
            nc.sync.dma_start(wt0f[:, 3, :], w0f[3, :, :])
            xload(0, 1)
            xload(1, 0)
            xload(1, 1)
            wt0b = wload(w0b, K0, "wt0b", nc.sync)
            bt0b = bload(b0b, "bt0b", nc.sync)
            wt1b = wload(w1b, K1, "wt1b", nc.sync)
            bt1b = bload(b1b, "bt1b", nc.sync)
            wt1f = wload(w1f, K1, "wt1f", nc.sync)
            bt1f = bload(b1f, "bt1f", nc.sync)

            # y1 resident, one tile per s-tile: [p, b, kk, s] with kk the
            # layer-1 contraction chunk (fw: 0..3, bw: 4..7)
            y1s = [rpool.tile([P, BC, K1, ST], F16, name=f"y1s{si}")
                   for si in range(NS)]

            def direction_pass(layer, fw, wt, bt, kc, last=False, first=False):
                dir_off = 0 if fw else HC
                s_order = range(NS) if fw else range(NS - 1, -1, -1)
                for b in range(BC):
                    carry = [cpool.tile([P, 1], F16, name=f"cr{b}_{hh}")
                             for hh in range(HC)]
                    for si, s_idx in enumerate(s_order):
                        s0 = s_idx * ST
                        # the kernel's globally-first and -last s-tiles run in
                        # two 256-col halves: at the head, activations start
                        # ~1.3us earlier (PSUM warmup stall shrinks); at the
                        # tail, the final act->stt->scan->mul->dma chain after
                        # the last matmul is half as long
                        if (last and b == BC - 1 and si == NS - 1) or (
                                first and b == 0 and si == 0):
                            for hh in range(HC):
                                ps = [
                                    ppool.tile([P, ST], F32, tag=f"ps{g}",
                                               name=f"ps{g}", bufs=(3 if g < 2 else 2))
                                    for g in range(3)
                                ]
                                z = spool.tile([P, ST], F16, tag="z", name="z")
                                f_ = spool.tile([P, ST], F16, tag="f", name="f")
                                o = spool.tile([P, ST], F16, tag="o", name="o")
                                g_ = spool.tile([P, ST], F16, tag="g", name="g")
                                c = spool.tile([P, ST], F16, tag="c", name="c")
                                y = spool.tile([P, ST], F16, tag="y", name="y")
                                bcol = lambda gi: bt[:, gi * HC + hh : gi * HC + hh + 1]
                                for h2 in range(2):
                                    cs = slice(h2 * (ST // 2), (h2 + 1) * (ST // 2))
                                    for g in range(3):
                                        cols = slice(g * H + hh * P, g * H + (hh + 1) * P)
                                        for k in range(kc):
                                            mov = (xres[b][k][:, s0 + cs.start:s0 + cs.stop]
                                                   if layer == 0
                                                   else y1s[s_idx][:, b, k, cs])
                                            nc.tensor.matmul(
                                                ps[g][:, cs], wt[:, k, cols], mov,
                                                start=(k == 0), stop=(k == kc - 1),
                                            )
                                    nc.scalar.activation(z[:, cs], ps[0][:, cs], ACT.Tanh, bias=bcol(0))
                                    nc.scalar.activation(f_[:, cs], ps[1][:, cs], ACT.Sigmoid, bias=bcol(1))
                                    nc.scalar.activation(o[:, cs], ps[2][:, cs], ACT.Sigmoid, bias=bcol(2))
                                    nc.vector.scalar_tensor_tensor(
                                        g_[:, cs], f_[:, cs], 1.0, z[:, cs], ALU.subtract, ALU.mult
                                    )
                                    if h2 == 0:
                                        init = 0.0 if si == 0 else carry[hh][:]
                                    else:
                                        init = c[:, cs.start - 1:cs.start]
                                    nc.vector.tensor_tensor_scan(
                                        c[:, cs], f_[:, cs], g_[:, cs], init,
                                        ALU.mult, ALU.subtract
                                    )
                                    if layer == 0:
                                        nc.gpsimd.tensor_mul(
                                            y1s[s_idx][:, b, dir_off + hh, cs],
                                            o[:, cs], c[:, cs]
                                        )
                                    else:
                                        nc.gpsimd.tensor_mul(y[:, cs], o[:, cs], c[:, cs])
                                        eng = nc.scalar if (b * NS + si) % 2 == 0 else nc.sync
                                        eng.dma_start(
                                            out_t[b, dir_off + hh, :, s0 + cs.start:s0 + cs.stop],
                                            y[:, cs]
                                        )
                                if si < NS - 1:
                                    nc.gpsimd.tensor_copy(
                                        carry[hh][:], c[:, ST - 1:ST]
                                    )
                            continue
                        for hh in range(HC):
                            ps = [
                                ppool.tile([P, ST], F32, tag=f"ps{g}",
                                           name=f"ps{g}", bufs=(3 if g < 2 else 2))
                                for g in range(3)
                            ]
                            for g in range(3):
                                cols = slice(g * H + hh * P, g * H + (hh + 1) * P)
                                for k in range(kc):
                                    mov = (xres[b][k][:, s0:s0 + ST] if layer == 0
                                           else y1s[s_idx][:, b, k, :])
                                    nc.tensor.matmul(
                                        ps[g][:], wt[:, k, cols], mov,
                                        start=(k == 0), stop=(k == kc - 1),
                                    )
                            z = spool.tile([P, ST], F16, tag="z", name="z")
                            f_ = spool.tile([P, ST], F16, tag="f", name="f")
                            o = spool.tile([P, ST], F16, tag="o", name="o")
                            g_ = spool.tile([P, ST], F16, tag="g", name="g")
                            c = spool.tile([P, ST], F16, tag="c", name="c")
                            bcol = lambda gi: bt[:, gi * HC + hh : gi * HC + hh + 1]
                            nc.scalar.activation(z[:], ps[0][:], ACT.Tanh, bias=bcol(0))
                            nc.scalar.activation(f_[:], ps[1][:], ACT.Sigmoid, bias=bcol(1))
                            nc.scalar.activation(o[:], ps[2][:], ACT.Sigmoid, bias=bcol(2))
                            # g' = (f-1)*z; scan: c = f*c - g' = f*c + (1-f)*z
                            nc.vector.scalar_tensor_tensor(
                                g_[:], f_[:], 1.0, z[:], ALU.subtract, ALU.mult
                            )
                            if fw:
                                sc = (c[:], f_[:], g_[:])
                                ccol = slice(ST - 1, ST)
                            else:
                                sc = (c[:, ::-1], f_[:, ::-1], g_[:, ::-1])
                                ccol = slice(0, 1)
                            init = 0.0 if si == 0 else carry[hh][:]
                            nc.vector.tensor_tensor_scan(
                                sc[0], sc[1], sc[2], init, ALU.mult, ALU.subtract
                            )
                            if si < NS - 1:
                                nc.gpsimd.tensor_copy(carry[hh][:], c[:, ccol])
                            if layer == 0:
                                nc.gpsimd.tensor_mul(
                                    y1s[s_idx][:, b, dir_off + hh, :], o[:], c[:]
                                )
                            else:
                                y = spool.tile([P, ST], F16, tag="y", name="y")
                                nc.gpsimd.tensor_mul(y[:], o[:], c[:])
                                # split out-writes across both HWDGE queues
                                # (per-queue effective DMA rate is well below
                                # the 358GB/s aggregate; one queue backlogs).
                                # Parity by s-tile, chosen so the final
                                # s-tiles land on sync -- scalar-queue issues
                                # must not delay the last activations.
                                eng = nc.scalar if (b * NS + si) % 2 == 0 else nc.sync
                                eng.dma_start(
                                    out_t[b, dir_off + hh, :, s0:s0 + ST], y[:]
                                )

            direction_pass(0, True, wt0f, bt0f, K0, first=True)
            direction_pass(0, False, wt0b, bt0b, K0)
            direction_pass(1, False, wt1b, bt1b, K1)
            direction_pass(1, True, wt1f, bt1f, K1, last=True)

    nc.finalize()
    return nc


_NC_CACHE = {}


def _get_nc(variant="fp16"):
    if variant not in _NC_CACHE:
        _NC_CACHE[variant] = build_nc()
    return _NC_CACHE[variant]


def kernel(X, seqlens, W_fw0, b_fw0, W_bw0, b_bw0, W_fw1, b_fw1, W_bw1, b_bw1,
           mm_dtype="fp16", trace=False):
    """Full-input entry point: shards over 8 cores, returns [B, S, 2H] f32."""
    del seqlens  # unused by the reference computation
    X = np.asarray(X, dtype=np.float32)

    def wprep(w, kc):  # [Din, 3H] f32 -> [kc, P, 3H] fp16
        return np.ascontiguousarray(
            np.asarray(w, np.float32).reshape(kc, P, 3 * H).astype(np.float16)
        )

    def bprep(b):  # [3H] f32 -> [P, 12] f32 (partition-major bias table)
        return np.ascontiguousarray(
            np.asarray(b, np.float32).reshape(3 * HC, P).T
        )

    weights = {
        "w0f": wprep(W_fw0, K0), "w0b": wprep(W_bw0, K0),
        "w1f": wprep(W_fw1, K1), "w1b": wprep(W_bw1, K1),
        "b0f": bprep(b_fw0), "b0b": bprep(b_bw0),
        "b1f": bprep(b_fw1), "b1b": bprep(b_bw1),
    }

    nc = _get_nc(mm_dtype)
    in_maps = []
    for i in range(N_CORES):
        rows = X[i * BC : (i + 1) * BC]  # [BC, S, D]
        xt_i = np.ascontiguousarray(
            rows.transpose(0, 2, 1).reshape(BC, K0, P, S).astype(np.float16)
        )
        in_maps.append({"xt": xt_i, **weights})

    res = bass_utils.run_bass_kernel_spmd(
        nc, in_maps, core_ids=list(range(N_CORES)), trace=trace
    )
    out = np.empty((B, S, 2 * H), dtype=np.float32)
    for i in range(N_CORES):
        out_t = res.results[i]["out_t"]  # [BC, 2*HC, P, S] fp16
        out[i * BC : (i + 1) * BC] = (
            out_t.reshape(BC, 2 * H, S).transpose(0, 2, 1).astype(np.float32)
        )
    kernel.last_results = res
    return out


# revision 20
# speedup vs baseline: 1.0057x; 1.0057x over previous
"""Bass/Trainium2 kernel for a 2-layer bidirectional QRNN (fo-pooling).

Reference computation (per layer, per direction):
    ZFO = X @ W + b            # [S, B, 3H]
    Z, F, O = split(ZFO); Z = tanh(Z); F = sigmoid(F); O = sigmoid(O)
    c_t = F_t * c_{t-1} + (1 - F_t) * Z_t        (bw direction: reversed time)
    Y_dir = O * C
    Y = concat(Y_fw, Y_bw)     # [S, B, 2H]
Two stacked layers; output is [B, S, 2H].

Sharding: data-parallel over batch. B=16 rows -> 2 rows per NeuronCore x 8.
Each core runs both layers for its 2 rows; no collectives.

v2 design (635us -> target ~PE roofline):
- All matmul operands are fp16 (1 cyc/row on the PE, same as fp32r, but
  2-byte LDWEIGHTS and no compute-producer restriction: weights and X are
  host-pre-cast and DMA'd directly; the fp32r staging/cast machinery and the
  192 DVE input casts of v1 are gone). Whole-pipeline fp16 rel-err vs the
  fp32 reference is ~1.4e-3 (CPU-simulated exactly).
- X ([BC,D,S] fp16, 4MB/core) and the layer-0 output y1 ([BC,2H,S] fp16,
  8MB/core) are SBUF-resident; y1 never round-trips DRAM. Total HBM traffic
  drops ~103MB -> ~21MB/core, and DMA issues from 348 -> ~80.
- Gate combine fused: one DVE scalar_tensor_tensor computes g' = (f-1)*z and
  the DVE scan runs state = f*state - g' = f*state + (1-f)*z, so the
  elementwise work per [128,512] tile is 3 scalar activations + 2 DVE ops +
  1 gpsimd mul -- all hidden under the 12 (layer0) / 24 (layer1) matmuls.
- Pass order l0-fw, l0-bw, l1-bw, l1-fw: l1-bw consumes y1 s-tiles in the
  order l0-bw produces them (descending), and l1-fw's first s-tile needs
  l0-bw's last -- so the PE queue never waits at a pass boundary.
"""

import numpy as np

import concourse.bacc as bacc
import concourse.mybir as mybir
from concourse import bass_utils
from concourse.tile import TileContext

# problem dims (hardcoded per spec)
B, S, D, H = 16, 2048, 512, 512
N_CORES = 8
BC = B // N_CORES  # batch rows per core
P = 128            # SBUF partitions
ST = 512           # s-tile (max moving free dim)
NS = S // ST       # 4 s-tiles
HC = H // P        # 4 hidden chunks
K0 = D // P        # 4 contraction chunks, layer 0
K1 = 2 * H // P    # 8 contraction chunks, layer 1

F32 = mybir.dt.float32
F16 = mybir.dt.float16
ACT = mybir.ActivationFunctionType
ALU = mybir.AluOpType


def build_nc():
    """Build the SPMD Bass program (same program on every core)."""
    nc = bacc.Bacc("TRN2", target_bir_lowering=False)

    xt = nc.dram_tensor("xt", [BC, K0, P, S], F16, kind="ExternalInput")
    w0f = nc.dram_tensor("w0f", [K0, P, 3 * H], F16, kind="ExternalInput")
    w0b = nc.dram_tensor("w0b", [K0, P, 3 * H], F16, kind="ExternalInput")
    w1f = nc.dram_tensor("w1f", [K1, P, 3 * H], F16, kind="ExternalInput")
    w1b = nc.dram_tensor("w1b", [K1, P, 3 * H], F16, kind="ExternalInput")
    # biases host-prearranged to [P, 12] so the load is a plain 2D DMA (the
    # "(j p) -> p j" gather was 4-byte-element descriptors, ~11.5us!)
    b0f = nc.dram_tensor("b0f", [P, 3 * HC], F32, kind="ExternalInput")
    b0b = nc.dram_tensor("b0b", [P, 3 * HC], F32, kind="ExternalInput")
    b1f = nc.dram_tensor("b1f", [P, 3 * HC], F32, kind="ExternalInput")
    b1b = nc.dram_tensor("b1b", [P, 3 * HC], F32, kind="ExternalInput")
    # out rows indexed (dir*HC + hh)*P + p -> feature dir*H + hh*P + p
    out_t = nc.dram_tensor("out_t", [BC, 2 * HC, P, S], F16, kind="ExternalOutput")

    with TileContext(nc) as tc:
        with (
            tc.tile_pool(name="res", bufs=1) as rpool,
            tc.tile_pool(name="scr", bufs=3) as spool,
            tc.tile_pool(name="carry", bufs=1) as cpool,
            tc.tile_pool(name="ps", bufs=1, space="PSUM") as ppool,
        ):
            # ---- resident loads: X rides the sync HWDGE queue, weights +
            # biases the scalar HWDGE queue, so the two DMA streams transfer
            # in parallel and the first matmul is gated by max(X[b0,s-half0],
            # w0f) instead of their sum. Per-k chunks (contiguous 2D
            # patterns, cheap ~0.6us issues); X additionally split into
            # s-halves so iteration 0 only waits for the first half. The
            # scalar queue's issue burst (28 x 0.6us) finishes before the
            # first activation needs it. ----
            def wload(wd, kc, name, eng):
                wt = rpool.tile([P, kc, 3 * H], F16, name=name)
                for k in range(kc):
                    eng.dma_start(wt[:, k, :], wd[k, :, :])
                return wt

            btiles = {n: rpool.tile([P, 3 * HC], F32, name=n)
                      for n in ("bt0f", "bt0b", "bt1f", "bt1b")}

            def bload(bd, name, eng):
                bt = btiles[name]
                eng.dma_start(bt[:], bd[:, :])
                return bt

            xres = [[rpool.tile([P, S], F16, name=f"x{b}k{k}") for k in range(K0)]
                    for b in range(BC)]

            def xload(b, half):
                sl = slice(0, S // 2) if half == 0 else slice(S // 2, S)
                for k in range(K0):
                    nc.sync.dma_start(xres[b][k][:, sl], xt[b, k, :, sl])

            # The first matmul iteration is gated on X[b0, s-half0] + all of
            # w0f: balance those ~2.5MB across BOTH HWDGE queues so they
            # transfer in parallel (~4us), then keep the scalar queue clear
            # for activations. Everything else queues on sync behind X --
            # transferred long before use (w0b by ~35us vs first use ~140us),
            # and out-writes only start at layer 1 (~300us).
            wt0f = rpool.tile([P, K0, 3 * H], F16, name="wt0f")
            nc.scalar.dma_start(wt0f[:, 0, :], w0f[0, :, :])
            nc.scalar.dma_start(wt0f[:, 1, :], w0f[1, :, :])
            bt0f = bload(b0f, "bt0f", nc.scalar)
            # warmup activations: force BOTH act-table loads (tanh+sigmoid,
            # 1.3us each) onto the scalar queue now, during the X-transfer
            # window -- otherwise the sigmoid table loads lazily between the
            # first real tanh and sigmoid. Outputs land in bt0b, which the
            # (later-issued) b0b DMA fully overwrites.
            nc.scalar.activation(btiles["bt0b"][:, 0:1], bt0f[:, 0:1], ACT.Tanh)
            nc.scalar.activation(btiles["bt0b"][:, 1:2], bt0f[:, 0:1], ACT.Sigmoid)
            xload(0, 0)
            nc.sync.dma_start(wt0f[:, 2, :], w0f[2, :, :])
            nc.sync.dma_start(wt0f[:, 3, :], w0f[3, :, :])
            xload(0, 1)
            xload(1, 0)
            xload(1, 1)
            wt0b = wload(w0b, K0, "wt0b", nc.sync)
            bt0b = bload(b0b, "bt0b", nc.sync)
            wt1b = wload(w1b, K1, "wt1b", nc.sync)
            bt1b = bload(b1b, "bt1b", nc.sync)
            wt1f = wload(w1f, K1, "wt1f", nc.sync)
            bt1f = bload(b1f, "bt1f", nc.sync)

            # y1 resident, one tile per s-tile: [p, b, kk, s] with kk the
            # layer-1 contraction chunk (fw: 0..3, bw: 4..7)
            y1s = [rpool.tile([P, BC, K1, ST], F16, name=f"y1s{si}")
                   for si in range(NS)]

            def direction_pass(layer, fw, wt, bt, kc, last=False, first=False):
                dir_off = 0 if fw else HC
                s_order = range(NS) if fw else range(NS - 1, -1, -1)
                for b in range(BC):
                    carry = [cpool.tile([P, 1], F16, name=f"cr{b}_{hh}")
                             for hh in range(HC)]
                    for si, s_idx in enumerate(s_order):
                        s0 = s_idx * ST
                        # the kernel's globally-first and -last s-tiles run in
                        # two 256-col halves: at the head, activations start
                        # ~1.3us earlier (PSUM warmup stall shrinks); at the
                        # tail, the final act->stt->scan->mul->dma chain after
                        # the last matmul is half as long
                        if (last and b == BC - 1 and si == NS - 1) or (
                                first and b == 0 and si == 0):
                            for hh in range(HC):
                                ps = [
                                    ppool.tile([P, ST], F32, tag=f"ps{g}",
                                               name=f"ps{g}", bufs=(3 if g < 2 else 2))
                                    for g in range(3)
                                ]
                                z = spool.tile([P, ST], F16, tag="z", name="z")
                                f_ = spool.tile([P, ST], F16, tag="f", name="f")
                                o = spool.tile([P, ST], F16, tag="o", name="o")
                                g_ = spool.tile([P, ST], F16, tag="g", name="g")
                                c = spool.tile([P, ST], F16, tag="c", name="c")
                                y = spool.tile([P, ST], F16, tag="y", name="y")
                                bcol = lambda gi: bt[:, gi * HC + hh : gi * HC + hh + 1]
                                for h2 in range(2):
                                    cs = slice(h2 * (ST // 2), (h2 + 1) * (ST // 2))
                                    for g in range(3):
                                        cols = slice(g * H + hh * P, g * H + (hh + 1) * P)
                                        for k in range(kc):
                                            mov = (xres[b][k][:, s0 + cs.start:s0 + cs.stop]
                                                   if layer == 0
                                                   else y1s[s_idx][:, b, k, cs])
                                            nc.tensor.matmul(
                                                ps[g][:, cs], wt[:, k, cols], mov,
                                                start=(k == 0), stop=(k == kc - 1),
                                            )
                                    nc.scalar.activation(z[:, cs], ps[0][:, cs], ACT.Tanh, bias=bcol(0))
                                    nc.scalar.activation(f_[:, cs], ps[1][:, cs], ACT.Sigmoid, bias=bcol(1))
                                    nc.scalar.activation(o[:, cs], ps[2][:, cs], ACT.Sigmoid, bias=bcol(2))
                                    nc.vector.scalar_tensor_tensor(
                                        g_[:, cs], f_[:, cs], 1.0, z[:, cs], ALU.subtract, ALU.mult
                                    )
                                    if h2 == 0:
                                        init = 0.0 if si == 0 else carry[hh][:]
                                    else:
                                        init = c[:, cs.start - 1:cs.start]
                                    nc.vector.tensor_tensor_scan(
                                        c[:, cs], f_[:, cs], g_[:, cs], init,
                                        ALU.mult, ALU.subtract
                                    )
                                    if layer == 0:
                                        nc.gpsimd.tensor_mul(
                                            y1s[s_idx][:, b, dir_off + hh, cs],
                                            o[:, cs], c[:, cs]
                                        )
                                    else:
                                        nc.gpsimd.tensor_mul(y[:, cs], o[:, cs], c[:, cs])
                                        eng = nc.scalar if (b * NS + si) % 2 == 0 else nc.sync
                                        eng.dma_start(
                                            out_t[b, dir_off + hh, :, s0 + cs.start:s0 + cs.stop],
                                            y[:, cs]
                                        )
                                if si < NS - 1:
                                    nc.gpsimd.tensor_copy(
                                        carry[hh][:], c[:, ST - 1:ST]
                                    )
                            continue
                        for hh in range(HC):
                            ps = [
                                ppool.tile([P, ST], F32, tag=f"ps{g}",
                                           name=f"ps{g}", bufs=(3 if g < 2 else 2))
                                for g in range(3)
                            ]
                            for g in range(3):
                                cols = slice(g * H + hh * P, g * H + (hh + 1) * P)
                                for k in range(kc):
                                    mov = (xres[b][k][:, s0:s0 + ST] if layer == 0
                                           else y1s[s_idx][:, b, k, :])
                                    nc.tensor.matmul(
                                        ps[g][:], wt[:, k, cols], mov,
                                        start=(k == 0), stop=(k == kc - 1),
                                    )
                            z = spool.tile([P, ST], F16, tag="z", name="z")
                            f_ = spool.tile([P, ST], F16, tag="f", name="f")
                            o = spool.tile([P, ST], F16, tag="o", name="o")
                            g_ = spool.tile([P, ST], F16, tag="g", name="g")
                            c = spool.tile([P, ST], F16, tag="c", name="c")
                            bcol = lambda gi: bt[:, gi * HC + hh : gi * HC + hh + 1]
                            nc.scalar.activation(z[:], ps[0][:], ACT.Tanh, bias=bcol(0))
                            nc.scalar.activation(f_[:], ps[1][:], ACT.Sigmoid, bias=bcol(1))
                            nc.scalar.activation(o[:], ps[2][:], ACT.Sigmoid, bias=bcol(2))
                            # g' = (f-1)*z; scan: c = f*c - g' = f*c + (1-f)*z
                            nc.vector.scalar_tensor_tensor(
                                g_[:], f_[:], 1.0, z[:], ALU.subtract, ALU.mult
                            )
                            if fw:
                                sc = (c[:], f_[:], g_[:])
                                ccol = slice(ST - 1, ST)
                            else:
                                sc = (c[:, ::-1], f_[:, ::-1], g_[:, ::-1])
                                ccol = slice(0, 1)
                            init = 0.0 if si == 0 else carry[hh][:]
                            nc.vector.tensor_tensor_scan(
                                sc[0], sc[1], sc[2], init, ALU.mult, ALU.subtract
                            )
                            if si < NS - 1:
                                nc.gpsimd.tensor_copy(carry[hh][:], c[:, ccol])
                            if layer == 0:
                                nc.gpsimd.tensor_mul(
                                    y1s[s_idx][:, b, dir_off + hh, :], o[:], c[:]
                                )
                            else:
                                y = spool.tile([P, ST], F16, tag="y", name="y")
                                nc.gpsimd.tensor_mul(y[:], o[:], c[:])
                                # split out-writes across both HWDGE queues
                                # (per-queue effective DMA rate is well below
                                # the 358GB/s aggregate; one queue backlogs).
                                # Parity by s-tile, chosen so the final
                                # s-tiles land on sync -- scalar-queue issues
                                # must not delay the last activations.
                                # last pass + last row stays entirely on sync:
                                # scalar issues there would delay the final acts
                                eng = (nc.sync if (last and b == BC - 1)
                                       else nc.scalar if (b * NS + si) % 2 == 0
                                       else nc.sync)
                                eng.dma_start(
                                    out_t[b, dir_off + hh, :, s0:s0 + ST], y[:]
                                )

            direction_pass(0, True, wt0f, bt0f, K0, first=True)
            direction_pass(0, False, wt0b, bt0b, K0)
            direction_pass(1, False, wt1b, bt1b, K1)
            direction_pass(1, True, wt1f, bt1f, K1, last=True)

    nc.finalize()
    return nc


_NC_CACHE = {}


def _get_nc(variant="fp16"):
    if variant not in _NC_CACHE:
        _NC_CACHE[variant] = build_nc()
    return _NC_CACHE[variant]


def kernel(X, seqlens, W_fw0, b_fw0, W_bw0, b_bw0, W_fw1, b_fw1, W_bw1, b_bw1,
           mm_dtype="fp16", trace=False):
    """Full-input entry point: shards over 8 cores, returns [B, S, 2H] f32."""
    del seqlens  # unused by the reference computation
    X = np.asarray(X, dtype=np.float32)

    def wprep(w, kc):  # [Din, 3H] f32 -> [kc, P, 3H] fp16
        return np.ascontiguousarray(
            np.asarray(w, np.float32).reshape(kc, P, 3 * H).astype(np.float16)
        )

    def bprep(b):  # [3H] f32 -> [P, 12] f32 (partition-major bias table)
        return np.ascontiguousarray(
            np.asarray(b, np.float32).reshape(3 * HC, P).T
        )

    weights = {
        "w0f": wprep(W_fw0, K0), "w0b": wprep(W_bw0, K0),
        "w1f": wprep(W_fw1, K1), "w1b": wprep(W_bw1, K1),
        "b0f": bprep(b_fw0), "b0b": bprep(b_bw0),
        "b1f": bprep(b_fw1), "b1b": bprep(b_bw1),
    }

    nc = _get_nc(mm_dtype)
    in_maps = []
    for i in range(N_CORES):
        rows = X[i * BC : (i + 1) * BC]  # [BC, S, D]
        xt_i = np.ascontiguousarray(
            rows.transpose(0, 2, 1).reshape(BC, K0, P, S).astype(np.float16)
        )
        in_maps.append({"xt": xt_i, **weights})

    res = bass_utils.run_bass_kernel_spmd(
        nc, in_maps, core_ids=list(range(N_CORES)), trace=trace
    )
    out = np.empty((B, S, 2 * H), dtype=np.float32)
    for i in range(N_CORES):
        out_t = res.results[i]["out_t"]  # [BC, 2*HC, P, S] fp16
        out[i * BC : (i + 1) * BC] = (
            out_t.reshape(BC, 2 * H, S).transpose(0, 2, 1).astype(np.float32)
        )
    kernel.last_results = res
    return out
